# revision 17
# baseline (speedup 1.0000x reference)
"""Bass/Trainium2 kernel for nn_BitGatConv (GAT-style message passing).

Self-contained: takes full inputs, shards edges by destination window across
8 NeuronCores (SPMD, one program), returns the full [N, HC] output.

v2 design (per core, window-dealt + rotated node ids so all cores run the
same program):
  Host prep: windows (64 dst nodes each) are sorted by edge count and dealt
    round-robin to (slot, core) so every core's slot s has a similar count;
    the node ids are permuted accordingly (output unpermuted at the end).
    Edges are binned per (core, slot) into k_s bins of 128 edge slots
    (k_s shared across cores = max need). Each bin's edges gather from one
    of two overlapping table halves (rows 0..32767 / rows 17408..50175) so
    int16 gather indices reach the whole 50176-row table; edges whose src
    falls in the 15360-row overlap are assigned to whichever half aligns
    the per-slot split to a bin boundary (minimal padding).
  Phase A (build): h = nodes_ft @ W, att_j = nodes_ft @ (W@A2) stored to a
    DRAM table hj_table [N_PAD, 128] rows = [h | att_j] (bf16);
    att_i = nodes_ft @ (W@A1) for the core's own shard kept in SBUF
    (aw [64, NPC, 64]).
  Phase B (edges), per group of GW slots:
    - dma_gather hj rows by src (the ONLY per-edge HBM gather: 256B/edge)
    - build onehot [128e, nb, 64n] on-chip: is_equal(tgt bcast, iota row)
    - build onehotT [64n, nb*128e]: gpsimd partition_broadcast of the
      bin-major tgt stream, then is_equal vs per-partition iota column
    - att_i per edge via PE: psum[e, ch] = onehotT^T @ aw_slot
    - s = att_j + att_i; l = max(0.2 s, s); x = exp(l) (shift-free softmax:
      logits are bounded so exp is safe); y = x * h
    - scatter via PE: pair psum [128, 128] accumulates onehot^T-weighted
      [y | x] per window over the slot's k_s bins (v1-proven tile_position
      packing of two windows per psum tile)
  Phase C (flush): out = numer / (denom + 1e-16) + bias -> [NSHARD, HC];
    host inverts the window permutation.
"""

import math
import os
import sys
from contextlib import ExitStack

import numpy as np

for _p in ("/opt/trn_rl_repo",):
    if _p not in sys.path:
        sys.path.insert(0, _p)

import ml_dtypes  # noqa: E402

BF16_NP = ml_dtypes.bfloat16

# ---------------------------------------------------------------------------
# Problem constants (hardcoded per contest rules)
N_NODES = 50000
N_EDGES = 800000
IN_CH = 128
HC = 64
NEG_SLOPE = 0.2
N_CORES = 8
W_WIN = 64  # nodes per scatter window (one-hot width)


def _cfg(n_nodes=N_NODES, n_edges=N_EDGES, n_cores=N_CORES, w=W_WIN):
    nw = math.ceil(n_nodes / w)
    npc = math.ceil(nw / n_cores)  # window slots per core
    if npc % 2 == 1:
        npc += 1  # flush in pairs
    n_pad = n_cores * npc * w
    nshard = npc * w
    gw = 1
    for cand in (7, 6, 5, 4, 8, 3, 2):
        if npc % cand == 0:
            gw = cand
            break
    return dict(
        N=n_nodes, E=n_edges, NC=n_cores, W=w, NPC=npc,
        N_PAD=n_pad, NSHARD=nshard, GW=gw, NGROUPS=npc // gw,
        NPAIR=npc // 2, T_TILES=n_pad // 128, SHARD_TILES=nshard // 128,
        BASE1=n_pad - 32768, HI_LIM=32768,
    )


def _prep(inputs, cfg):
    """Host-side preprocessing: permutation, binning, index streams."""
    N, E, NC, W = cfg["N"], cfg["E"], cfg["NC"], cfg["W"]
    NPC, N_PAD, NSHARD = cfg["NPC"], cfg["N_PAD"], cfg["NSHARD"]
    GW, NGROUPS, NPAIR = cfg["GW"], cfg["NGROUPS"], cfg["NPAIR"]
    BASE1, HI_LIM = cfg["BASE1"], cfg["HI_LIM"]
    NW = N_PAD // W

    nodes_ft = np.asarray(inputs["nodes_ft"], dtype=np.float32)
    adj = np.asarray(inputs["adj_list"])
    weight = np.asarray(inputs["weight"], dtype=np.float32)
    a1 = np.asarray(inputs["att_layer_1"], dtype=np.float32)
    a2 = np.asarray(inputs["att_layer_2"], dtype=np.float32)
    bias = np.asarray(inputs["bias"], dtype=np.float32)

    tgt0 = adj[0].astype(np.int64)
    src0 = adj[1].astype(np.int64)

    # ---- window dealing permutation
    cnt_old = np.bincount(tgt0 // W, minlength=NW)
    order_w = np.argsort(-cnt_old, kind="stable")
    new_of_oldwin = np.empty(NW, dtype=np.int64)
    idxs = np.arange(NW)
    new_of_oldwin[order_w] = (idxs % NC) * NPC + idxs // NC
    perm = new_of_oldwin[np.arange(N_PAD) // W] * W + np.arange(N_PAD) % W

    tgt = perm[tgt0]
    src = perm[src0]

    # ---- per-edge core/slot/class
    c_e = tgt // NSHARD
    s_e = (tgt // W) % NPC
    tloc = (tgt % W).astype(np.float32)
    src_rot = (src - c_e * NSHARD) % N_PAD
    f_lo = src_rot < BASE1
    f_hi = src_rot >= HI_LIM
    f_fr = ~f_lo & ~f_hi

    key = c_e * NPC + s_e
    cnt_cs = np.bincount(key, minlength=NC * NPC).reshape(NC, NPC)
    flo_cs = np.bincount(key[f_lo], minlength=NC * NPC).reshape(NC, NPC)
    fhi_cs = np.bincount(key[f_hi], minlength=NC * NPC).reshape(NC, NPC)

    maxcnt = cnt_cs.max(axis=0)
    maxflo = flo_cs.max(axis=0)
    maxfhi = fhi_cs.max(axis=0)
    assert maxcnt.min() > 0, "empty slot: need dummy handling"

    lob = np.ceil(maxflo / 128).astype(np.int64)
    k_s = np.maximum(np.ceil(maxcnt / 128).astype(np.int64),
                     lob + np.ceil(maxfhi / 128).astype(np.int64))
    hib = k_s - lob
    assert (flo_cs <= lob[None, :] * 128).all()
    assert (fhi_cs <= hib[None, :] * 128).all()

    # per-(core,slot) lo occupancy
    nfree = cnt_cs - flo_cs - fhi_cs
    lo_w = np.maximum(np.minimum(flo_cs + nfree, lob[None, :] * 128),
                      cnt_cs - hib[None, :] * 128)
    quota = lo_w - flo_cs  # free edges sent to lo

    # free-edge rank within (c, s)
    fr_idx = np.flatnonzero(f_fr)
    fr_ord = fr_idx[np.argsort(key[fr_idx], kind="stable")]
    fr_cnt = np.bincount(key[fr_idx], minlength=NC * NPC)
    fr_starts = np.zeros(NC * NPC + 1, dtype=np.int64)
    fr_starts[1:] = np.cumsum(fr_cnt)
    fr_rank = np.empty(E, dtype=np.int64)
    fr_rank[fr_ord] = np.arange(len(fr_ord)) - fr_starts[key[fr_ord]]

    is_lo = f_lo.copy()
    is_lo[f_fr] = fr_rank[f_fr] < quota.reshape(-1)[key[f_fr]]

    # ranks within (c, s, class)
    tloc_i = (tgt % W).astype(np.int64)

    def class_rank(mask, sec=None):
        mi = np.flatnonzero(mask)
        if sec is None:
            order = np.argsort(key[mi], kind="stable")
        else:
            order = np.lexsort((sec[mi], key[mi]))
        mo = mi[order]
        mc = np.bincount(key[mi], minlength=NC * NPC)
        ms = np.zeros(NC * NPC + 1, dtype=np.int64)
        ms[1:] = np.cumsum(mc)
        r = np.empty(len(mi), dtype=np.int64)
        r[order] = np.arange(len(mi)) - ms[key[mo]]
        out = np.zeros(E, dtype=np.int64)
        out[mi] = r
        return out

    # first pass: arbitrary order -> bin assignment j
    rank_lo = class_rank(is_lo)
    rank_hi = class_rank(~is_lo)
    j_tmp = np.where(is_lo, rank_lo // 128, rank_hi // 128)
    # second pass: within each bin, order by window-local tgt (staircase)
    sec = j_tmp * 64 + tloc_i
    rank_lo = class_rank(is_lo, sec)
    rank_hi = class_rank(~is_lo, sec)

    # stream bases (shared across cores)
    lo_base = np.zeros(NPC, dtype=np.int64)
    lo_base[1:] = np.cumsum(lob)[:-1]
    hi_base = np.zeros(NPC, dtype=np.int64)
    hi_base[1:] = np.cumsum(hib)[:-1]
    LOB = int(lob.sum())
    HIB = int(hib.sum())
    B = int(k_s.sum())

    # group structures
    grp_slots = [list(range(g * GW, (g + 1) * GW)) for g in range(NGROUPS)]
    nlo_g = [int(lob[s0:s0 + GW].sum()) for s0 in range(0, NPC, GW)]
    nhi_g = [int(hib[s0:s0 + GW].sum()) for s0 in range(0, NPC, GW)]
    nb_g = [a + b for a, b in zip(nlo_g, nhi_g)]
    grpoff = np.zeros(NGROUPS + 1, dtype=np.int64)
    grpoff[1:] = np.cumsum(nb_g)

    # processed-order bin id for each (slot, class, j)
    g_of_s = np.arange(NPC) // GW
    pb_lo_s = grpoff[g_of_s] + (lo_base - lo_base[g_of_s * GW])
    pb_hi_s = (grpoff[g_of_s] + np.array(nlo_g)[g_of_s]
               + (hi_base - hi_base[g_of_s * GW]))

    # per-edge stream positions
    j_e = np.where(is_lo, rank_lo // 128, rank_hi // 128)
    p_e = np.where(is_lo, rank_lo % 128, rank_hi % 128)
    pb_e = np.where(is_lo, pb_lo_s[s_e] + j_e, pb_hi_s[s_e] + j_e)

    lo_s = np.zeros((NC, LOB * 128), dtype=np.int16)
    m = is_lo
    lo_s[c_e[m], (lo_base[s_e[m]] + j_e[m]) * 128 + p_e[m]] = (
        src_rot[m].astype(np.int16))
    hi_s = np.zeros((NC, HIB * 128), dtype=np.int16)
    m = ~is_lo
    hi_s[c_e[m], (hi_base[s_e[m]] + j_e[m]) * 128 + p_e[m]] = (
        (src_rot[m] - BASE1).astype(np.int16))

    tgt_col = np.full((NC, 128, B), -1.0, dtype=np.float32)
    tgt_col[c_e, p_e, pb_e] = tloc

    # staircase thresholds: per (core, bin, n): [pads..., tloc asc]
    hist = np.zeros((NC, B, W), dtype=np.int64)
    np.add.at(hist, (c_e, pb_e, tloc_i), 1)
    cum = np.cumsum(hist, axis=2)
    thr_hi = cum.astype(np.float32)
    thr_lo = thr_hi - hist.astype(np.float32)
    thr_lo = np.ascontiguousarray(thr_lo.transpose(0, 2, 1))  # [NC, W, B]
    thr_hi = np.ascontiguousarray(thr_hi.transpose(0, 2, 1))

    def wrap16(stream2d):
        ncc, L = stream2d.shape
        w = stream2d.reshape(ncc, L // 16, 16).transpose(0, 2, 1)
        return np.ascontiguousarray(np.tile(w, (1, 8, 1)))

    lo_idx = wrap16(lo_s)
    hi_idx = wrap16(hi_s)

    # permuted, transposed, padded node features
    base = np.zeros((IN_CH, N_PAD), dtype=np.float32)
    base[:, perm[np.arange(N)]] = nodes_ft.T

    wh = weight.astype(BF16_NP)
    wi = (weight @ a1).astype(BF16_NP)
    wj = (weight @ a2).astype(BF16_NP)
    bias_full = np.tile(bias[None, :], (128, NPAIR)).astype(BF16_NP)

    in_maps = []
    for c in range(NC):
        nftT = np.ascontiguousarray(np.roll(base, -c * NSHARD, axis=1))
        in_maps.append({
            "nodes_ftT": nftT.astype(BF16_NP),
            "wh": wh, "wi": wi, "wj": wj,
            "lo_idx": lo_idx[c],
            "hi_idx": hi_idx[c],
            "tgt_col": tgt_col[c].astype(BF16_NP),
            "thr_lo": thr_lo[c],
            "thr_hi": thr_hi[c],
            "bias_bc": bias_full,
        })

    meta = dict(
        k_s=k_s.tolist(), lob=lob.tolist(), hib=hib.tolist(),
        LOB=LOB, HIB=HIB, B=B,
        nlo_g=nlo_g, nhi_g=nhi_g, nb_g=nb_g,
        NBMAX=max(nb_g), perm=perm,
    )
    return in_maps, meta


def _build_program(cfg, meta, debug_dump=False, phase_limit="full", repeat=1):
    import concourse.bacc as bacc
    import concourse.bass as bass
    import concourse.mybir as mybir
    import concourse.tile as tile
    from concourse.bass import AP

    BF16 = mybir.dt.bfloat16
    F32 = mybir.dt.float32
    I16 = mybir.dt.int16
    I32 = mybir.dt.int32
    ALU = mybir.AluOpType
    ACT = mybir.ActivationFunctionType

    NPC, N_PAD, NSHARD = cfg["NPC"], cfg["N_PAD"], cfg["NSHARD"]
    T_TILES, SHARD_TILES = cfg["T_TILES"], cfg["SHARD_TILES"]
    GW, NGROUPS, NPAIR = cfg["GW"], cfg["NGROUPS"], cfg["NPAIR"]
    BASE1 = cfg["BASE1"]
    W = cfg["W"]

    k_s, lob, hib = meta["k_s"], meta["lob"], meta["hib"]
    LOB, HIB, B = meta["LOB"], meta["HIB"], meta["B"]
    nlo_g, nhi_g, nb_g = meta["nlo_g"], meta["nhi_g"], meta["nb_g"]
    NBMAX = meta["NBMAX"]
    NLOMAX = max(nlo_g)
    NHIMAX = max(nhi_g)
    CHUNK = int(os.environ.get("GAT_CHUNK", "32"))  # bins per gather call
    GBUFS = int(os.environ.get("GAT_GBUFS", "3"))
    DEFER = bool(int(os.environ.get("GAT_DEFER", "1")))

    nc = bacc.Bacc("TRN2", target_bir_lowering=False, debug=False,
                   num_swdge_queues=4)

    nodes_ftT = nc.dram_tensor("nodes_ftT", [IN_CH, N_PAD], BF16,
                               kind="ExternalInput")
    wh_d = nc.dram_tensor("wh", [IN_CH, HC], BF16, kind="ExternalInput")
    wi_d = nc.dram_tensor("wi", [IN_CH, HC], BF16, kind="ExternalInput")
    wj_d = nc.dram_tensor("wj", [IN_CH, HC], BF16, kind="ExternalInput")
    loidx_d = nc.dram_tensor("lo_idx", [128, LOB * 8], I16,
                             kind="ExternalInput")
    hiidx_d = nc.dram_tensor("hi_idx", [128, HIB * 8], I16,
                             kind="ExternalInput")
    tgtcol_d = nc.dram_tensor("tgt_col", [128, B], BF16, kind="ExternalInput")
    thrlo_d = nc.dram_tensor("thr_lo", [W, B], F32, kind="ExternalInput")
    thrhi_d = nc.dram_tensor("thr_hi", [W, B], F32, kind="ExternalInput")
    bias_d = nc.dram_tensor("bias_bc", [128, NPAIR * HC], BF16,
                            kind="ExternalInput")
    out_d = nc.dram_tensor("out", [NSHARD, HC], F32, kind="ExternalOutput")

    hj_table = nc.dram_tensor("hj_table", [N_PAD, 2 * HC], BF16,
                              kind="Internal")

    do_build = phase_limit != "noop"
    do_gather = phase_limit in ("gather", "noatt", "nomm", "full")
    do_att = phase_limit in ("noatt", "nomm", "full")  # onehot/att builds
    do_dve = phase_limit in ("nomm", "full")
    do_mm = phase_limit == "full"

    with tile.TileContext(nc) as tc, ExitStack() as ctx:
        const_pool = ctx.enter_context(tc.tile_pool(name="const", bufs=1))
        b_in = ctx.enter_context(tc.tile_pool(name="b_in", bufs=4))
        b_ps = ctx.enter_context(tc.tile_pool(name="b_ps", bufs=2,
                                              space="PSUM"))
        b_st = ctx.enter_context(tc.tile_pool(name="b_st", bufs=4))
        idx_pool = ctx.enter_context(tc.tile_pool(name="idx", bufs=2))
        g_pool = ctx.enter_context(tc.tile_pool(name="gp", bufs=GBUFS))
        oh_pool = ctx.enter_context(tc.tile_pool(name="ohp", bufs=2))
        tr_pool = ctx.enter_context(tc.tile_pool(name="trp", bufs=3))
        s_pool = ctx.enter_context(tc.tile_pool(name="sp", bufs=1))
        att_ps = ctx.enter_context(tc.tile_pool(name="attps", bufs=2,
                                                space="PSUM"))
        mm_ps = ctx.enter_context(tc.tile_pool(name="mmps", bufs=4,
                                               space="PSUM"))
        fl_pool = ctx.enter_context(tc.tile_pool(name="fl", bufs=1))

        wh_sb = const_pool.tile([IN_CH, HC], BF16)
        nc.sync.dma_start(wh_sb[:], wh_d[:])
        wi_sb = const_pool.tile([IN_CH, HC], BF16)
        nc.sync.dma_start(wi_sb[:], wi_d[:])
        wj_sb = const_pool.tile([IN_CH, HC], BF16)
        nc.sync.dma_start(wj_sb[:], wj_d[:])
        bias_sb = const_pool.tile([128, NPAIR * HC], BF16)
        nc.sync.dma_start(bias_sb[:], bias_d[:])

        # iota constants
        iota_ri = const_pool.tile([128, W], I32)
        nc.gpsimd.iota(iota_ri[:], pattern=[[1, W]], base=0,
                       channel_multiplier=0)
        iota_row = const_pool.tile([128, W], BF16)
        nc.vector.tensor_copy(iota_row[:], iota_ri[:])
        iota_f64i = const_pool.tile([64, 128], I32)
        nc.gpsimd.iota(iota_f64i[:], pattern=[[1, 128]], base=0,
                       channel_multiplier=0)
        iota_f64 = const_pool.tile([64, 128], BF16)
        nc.vector.tensor_copy(iota_f64[:], iota_f64i[:])

        # att_i table for this core's shard, [64, NPC, HC]
        aw_sb = const_pool.tile([64, NPC, HC], BF16)

        def emit_once(rep):
            # ---- Phase A: hj table (h | att_j), batched 2 tiles / iter
            for t2 in range(T_TILES // 2 if do_build else 0):
                t = 2 * t2
                dmae = nc.sync if t2 % 2 == 0 else nc.scalar
                nf = b_in.tile([128, 2, 128], BF16, name="nf")
                dmae.dma_start(
                    nf[:].rearrange("p a b -> p (a b)"),
                    nodes_ftT[:, 128 * t:128 * (t + 2)])
                ps = b_ps.tile([128, 2, 2 * HC], F32, name="bps")
                for u in range(2):
                    nc.tensor.matmul(ps[:, u, 0:HC], nf[:, u, :], wh_sb[:],
                                     start=(u == 0), stop=False)
                    nc.tensor.matmul(ps[:, u, HC:2 * HC], nf[:, u, :],
                                     wj_sb[:], start=False, stop=(u == 1))
                st = b_st.tile([128, 2, 2 * HC], BF16, name="bst")
                if t2 % 2 == 0:
                    nc.vector.tensor_copy(st[:], ps[:])
                else:
                    nc.scalar.copy(st[:], ps[:])
                dmae.dma_start(
                    hj_table[128 * t:128 * (t + 2), :].rearrange(
                        "(a p) b -> p a b", p=128),
                    st[:])
            # att_i for own shard -> SBUF aw (both 64-col halves to part 0-63)
            for t in range(SHARD_TILES if do_build else 0):
                nf2 = b_in.tile([128, 128], BF16, tag="nf2", name="nf2")
                dmae = nc.scalar if t % 2 == 0 else nc.sync
                dmae.dma_start(nf2[:], nodes_ftT[:, 128 * t:128 * (t + 1)])
                ps2f = b_ps.tile([128, 2, 2 * HC], F32, tag="bps",
                                 name="bps2")
                ps2 = ps2f[0:64, :, 0:HC]
                nc.tensor.matmul(ps2[:, 0, :], nf2[:, 0:64], wi_sb[:],
                                 start=True, stop=True)
                nc.tensor.matmul(ps2[:, 1, :], nf2[:, 64:128], wi_sb[:],
                                 start=True, stop=True)
                if t % 2 == 0:
                    nc.scalar.copy(aw_sb[:, 2 * t:2 * t + 2, :], ps2[:])
                else:
                    nc.vector.tensor_copy(aw_sb[:, 2 * t:2 * t + 2, :],
                                          ps2[:])

            if int(os.environ.get("GAT_BARRIER", "0")):
                tc.strict_bb_all_engine_barrier()

            # ---- Phase B
            stage_n = fl_pool.tile([128, NPAIR * HC], F32, tag="sn",
                                   name="sn")
            stage_d = fl_pool.tile([128, NPAIR * HC], F32, tag="sd",
                                   name="sd")

            pair_tiles = {}
            done_slots = set()
            qn = 0
            lo_off = 0
            hi_off = 0
            boff = 0
            last_G = last_OH = last_TR = None
            prev_grp = None

            def emit_scatter(bins_p, OH_p, G_p):
                for bl in range(len(bins_p)):
                    s_bl, jj = bins_p[bl]
                    pr, half = s_bl // 2, s_bl % 2
                    if pr not in pair_tiles:
                        pair_tiles[pr] = mm_ps.tile(
                            [128, 2 * HC], F32, tag="pp", name=f"pp{pr}")
                    ps_t = pair_tiles[pr]
                    nc.tensor.matmul(
                        ps_t[HC * half:HC * half + HC, :],
                        OH_p[:, bl, :],
                        G_p[:, bl, :],
                        start=(jj == 0), stop=(jj == k_s[s_bl] - 1),
                        tile_position=(0, HC * half),
                        skip_group_check=True,
                    )
                    if jj == k_s[s_bl] - 1:
                        done_slots.add(s_bl)
                        if (s_bl ^ 1) in done_slots:
                            flush_pair(pr)

            def flush_pair(pr):
                ps_t = pair_tiles.pop(pr)
                nc.vector.tensor_copy(
                    stage_n[:, HC * pr:HC * (pr + 1)], ps_t[:, 0:HC])
                nc.vector.tensor_copy(
                    stage_d[:, HC * pr:HC * (pr + 1)], ps_t[:, HC:2 * HC])

            for g in range(NGROUPS if do_gather else 0):
                slots = list(range(g * GW, (g + 1) * GW))
                nlo, nhi, NB = nlo_g[g], nhi_g[g], nb_g[g]
                # bins in processed order: (slot, jj) jj in 0..k_s-1
                bins = []
                for s in slots:
                    bins += [(s, j) for j in range(lob[s])]
                for s in slots:
                    bins += [(s, lob[s] + j) for j in range(hib[s])]

                sl_t = idx_pool.tile([128, NLOMAX * 8], I16, tag="sl",
                                     name="sl")
                sl = sl_t[:, 0:nlo * 8]
                nc.sync.dma_start(sl,
                                  loidx_d[:, lo_off * 8:(lo_off + nlo) * 8])
                sh_t = idx_pool.tile([128, NHIMAX * 8], I16, tag="sh",
                                     name="sh")
                sh = sh_t[:, 0:nhi * 8]
                nc.sync.dma_start(sh,
                                  hiidx_d[:, hi_off * 8:(hi_off + nhi) * 8])
                tct_t = idx_pool.tile([128, NBMAX], BF16, tag="tc", name="tc")
                nc.scalar.dma_start(tct_t[:, 0:NB], tgtcol_d[:, boff:boff + NB])
                thl_t = idx_pool.tile([W, NBMAX], F32, tag="thl", name="thl")
                nc.scalar.dma_start(thl_t[:, 0:NB], thrlo_d[:, boff:boff + NB])
                thh_t = idx_pool.tile([W, NBMAX], F32, tag="thh", name="thh")
                nc.scalar.dma_start(thh_t[:, 0:NB], thrhi_d[:, boff:boff + NB])

                G_t = g_pool.tile([128, NBMAX, 2 * HC], BF16, tag="G",
                                  name="G")
                G = G_t[:, 0:NB, :]

                def chunked_gather(obase, table_ap, idx_tile, nbins):
                    nonlocal qn
                    parts = max(1, (nbins + CHUNK - 1) // CHUNK)
                    cuts = [nbins * i // parts for i in range(parts + 1)]
                    for a, b2 in zip(cuts[:-1], cuts[1:]):
                        if a == b2:
                            continue
                        nc.gpsimd.dma_gather(
                            out_ap=G_t[:, obase + a:obase + b2, :],
                            in_ap=table_ap,
                            idxs_ap=idx_tile[:, a * 8:b2 * 8],
                            num_idxs=(b2 - a) * 128,
                            num_idxs_reg=(b2 - a) * 128,
                            elem_size=2 * HC, queue_num=qn % 4,
                            single_packet=False,
                        )
                        qn += 1

                chunked_gather(0, hj_table[0:32768, :], sl, nlo)
                chunked_gather(nlo, hj_table[BASE1:N_PAD, :], sh, nhi)
                last_G = G

                if not do_att:
                    lo_off += nlo
                    hi_off += nhi
                    boff += NB
                    continue

                # onehot [128, NB, 64]
                OH_t = oh_pool.tile([128, NBMAX, W], BF16, tag="OH",
                                    name="OH")
                OH = OH_t[:, 0:NB, :]
                t0 = tct_t[:, 0:NB]
                in0 = AP(t0.tensor, t0.offset,
                         [list(t0.ap[0]), list(t0.ap[1]), [0, W]])
                i1 = iota_row[:]
                in1 = AP(i1.tensor, i1.offset,
                         [list(i1.ap[0]), [0, NB], list(i1.ap[1])])
                nc.vector.tensor_tensor(out=OH[:], in0=in0, in1=in1,
                                        op=ALU.is_equal)

                # onehotT [64, NB, 128] staircase: bins are tloc-sorted,
                # so row n of a bin is 1 on [thr_lo[n], thr_hi[n])
                TR_t = tr_pool.tile([64, NBMAX, 128], BF16, tag="TR",
                                    name="TR")
                TR = TR_t[:, 0:NB, :]
                last_OH, last_TR = OH, TR
                a_t = tr_pool.tile([64, 128], BF16, tag="sca", name="sca")
                for bl in range(NB if do_att else 0):
                    nc.vector.tensor_scalar(
                        out=a_t[:], in0=iota_f64[:],
                        scalar1=thl_t[:, bl:bl + 1], scalar2=None,
                        op0=ALU.is_ge)
                    nc.vector.scalar_tensor_tensor(
                        out=TR[:, bl, :], in0=iota_f64[:],
                        scalar=thh_t[:, bl:bl + 1], in1=a_t[:],
                        op0=ALU.is_lt, op1=ALU.mult)

                # att_i matmuls + s = att_j + att_i, 8 bins per psum bank
                S_t = s_pool.tile([128, NBMAX, HC], BF16, tag="S", name="S")
                S = S_t[:, 0:NB, :]
                if do_dve:
                    for c0 in range(0, NB, 8):
                        c1 = min(c0 + 8, NB)
                        ap_t = att_ps.tile([128, 8, HC], F32, tag="att",
                                           name="att")
                        for bl in range(c0, c1):
                            s_bl = bins[bl][0]
                            nc.tensor.matmul(
                                ap_t[:, bl - c0, :], TR[:, bl, :],
                                aw_sb[:, s_bl, :], start=True, stop=True)
                        nc.vector.tensor_tensor(
                            out=S[:, c0:c1, :], in0=G[:, c0:c1, HC:2 * HC],
                            in1=ap_t[:, 0:c1 - c0, :], op=ALU.add)
                    # leaky relu; x = exp; y = x * h
                    nc.vector.scalar_tensor_tensor(
                        out=S, in0=S, scalar=NEG_SLOPE, in1=S,
                        op0=ALU.mult, op1=ALU.max)
                    nc.scalar.activation(G[:, :, HC:2 * HC], S, ACT.Exp)
                    nc.vector.tensor_tensor(
                        out=G[:, :, 0:HC], in0=G[:, :, 0:HC],
                        in1=G[:, :, HC:2 * HC], op=ALU.mult)

                if do_mm:
                    if DEFER:
                        if prev_grp is not None:
                            emit_scatter(*prev_grp)
                        prev_grp = (bins, OH, G)
                    else:
                        emit_scatter(bins, OH, G)

                lo_off += nlo
                hi_off += nhi
                boff += NB

            if do_mm and prev_grp is not None:
                emit_scatter(*prev_grp)

            # ---- Phase C
            if not do_mm:
                nc.vector.memset(stage_n[:], 0.0)
                nc.vector.memset(stage_d[:], 1.0)
            assert not pair_tiles, f"unflushed pairs: {list(pair_tiles)}"
            nc.vector.tensor_scalar_add(stage_d[:], stage_d[:], 1e-16)
            nc.scalar.activation(stage_d[:], stage_d[:], ACT.Ln)
            nc.scalar.activation(stage_d[:], stage_d[:], ACT.Exp, scale=-1.0)
            nc.vector.tensor_tensor(out=stage_n[:], in0=stage_n[:],
                                    in1=stage_d[:], op=ALU.mult)
            nc.vector.tensor_tensor(out=stage_n[:], in0=stage_n[:],
                                    in1=bias_sb[:], op=ALU.add)

            out_view = out_d[:].rearrange("(pr p) c -> p pr c", p=128)
            st_view = stage_n[:].rearrange("p (pr c) -> p pr c", c=HC)
            nc.sync.dma_start(out_view, st_view)
            return last_G, last_OH, last_TR, stage_d

        for rep in range(repeat):
            outs = emit_once(rep)
            if repeat > 1:
                tc.strict_bb_all_engine_barrier()
        last_G, last_OH, last_TR, stage_d = outs

        if debug_dump:
            NB_l = nb_g[-1]
            dump_hj = nc.dram_tensor("dump_hj", [N_PAD, 2 * HC], BF16,
                                     kind="ExternalOutput")
            dump_aw = nc.dram_tensor("dump_aw", [64, NPC * HC], BF16,
                                     kind="ExternalOutput")
            dump_sd = nc.dram_tensor("dump_sd", [128, NPAIR * HC], F32,
                                     kind="ExternalOutput")
            dump_g = nc.dram_tensor("dump_g", [128, NB_l * 2 * HC], BF16,
                                    kind="ExternalOutput")
            dump_oh = nc.dram_tensor("dump_oh", [128, NB_l * W], BF16,
                                     kind="ExternalOutput")
            dump_tr = nc.dram_tensor("dump_tr", [64, NB_l * 128], BF16,
                                     kind="ExternalOutput")
            tc.strict_bb_all_engine_barrier()
            nc.sync.dma_start(dump_hj[:], hj_table[:])
            nc.sync.dma_start(dump_aw[:],
                              aw_sb[:].rearrange("p a b -> p (a b)"))
            nc.sync.dma_start(dump_sd[:], stage_d[:])
            nc.sync.dma_start(dump_g[:],
                              last_G[:].rearrange("p a b -> p (a b)"))
            nc.sync.dma_start(dump_oh[:],
                              last_OH[:].rearrange("p a b -> p (a b)"))
            nc.sync.dma_start(dump_tr[:],
                              last_TR[:].rearrange("p a b -> p (a b)"))

    nc.compile()
    return nc


def kernel(**inputs):
    cfg = _cfg()
    in_maps, meta = _prep(inputs, cfg)
    nc = _build_program(cfg, meta)

    from concourse import bass_utils
    res = bass_utils.run_bass_kernel_spmd(
        nc, in_maps, core_ids=list(range(cfg["NC"])),
        trace=bool(int(os.environ.get("GAT_TRACE", "0"))),
    )
    kernel.last_result = res
    kernel.last_ctx = (nc, in_maps, cfg, meta)

    NSHARD = cfg["NSHARD"]
    out_new = np.concatenate(
        [res.results[c]["out"] for c in range(cfg["NC"])], axis=0)
    perm = meta["perm"]
    return out_new[perm[:cfg["N"]]]


# revision 19
# speedup vs baseline: 1.1363x; 1.1363x over previous
"""Bass/Trainium2 kernel for nn_BitGatConv (GAT-style message passing).

Self-contained: takes full inputs, shards edges by destination window across
8 NeuronCores (SPMD, one program), returns the full [N, HC] output.

v2 design (per core, window-dealt + rotated node ids so all cores run the
same program):
  Host prep: windows (64 dst nodes each) are sorted by edge count and dealt
    round-robin to (slot, core) so every core's slot s has a similar count;
    the node ids are permuted accordingly (output unpermuted at the end).
    Edges are binned per (core, slot) into k_s bins of 128 edge slots
    (k_s shared across cores = max need). Each bin's edges gather from one
    of two overlapping table halves (rows 0..32767 / rows 17408..50175) so
    int16 gather indices reach the whole 50176-row table; edges whose src
    falls in the 15360-row overlap are assigned to whichever half aligns
    the per-slot split to a bin boundary (minimal padding).
  Phase A (build): h = nodes_ft @ W, att_j = nodes_ft @ (W@A2) stored to a
    DRAM table hj_table [N_PAD, 128] rows = [h | att_j] (bf16);
    att_i = nodes_ft @ (W@A1) for the core's own shard kept in SBUF
    (aw [64, NPC, 64]).
  Phase B (edges), per group of GW slots:
    - dma_gather hj rows by src (the ONLY per-edge HBM gather: 256B/edge)
    - build onehot [128e, nb, 64n] on-chip: is_equal(tgt bcast, iota row)
    - build onehotT [64n, nb*128e]: gpsimd partition_broadcast of the
      bin-major tgt stream, then is_equal vs per-partition iota column
    - att_i per edge via PE: psum[e, ch] = onehotT^T @ aw_slot
    - s = att_j + att_i; l = max(0.2 s, s); x = exp(l) (shift-free softmax:
      logits are bounded so exp is safe); y = x * h
    - scatter via PE: pair psum [128, 128] accumulates onehot^T-weighted
      [y | x] per window over the slot's k_s bins (v1-proven tile_position
      packing of two windows per psum tile)
  Phase C (flush): out = numer / (denom + 1e-16) + bias -> [NSHARD, HC];
    host inverts the window permutation.
"""

import math
import os
import sys
from contextlib import ExitStack

import numpy as np

for _p in ("/opt/trn_rl_repo",):
    if _p not in sys.path:
        sys.path.insert(0, _p)

import ml_dtypes  # noqa: E402

BF16_NP = ml_dtypes.bfloat16

# ---------------------------------------------------------------------------
# Problem constants (hardcoded per contest rules)
N_NODES = 50000
N_EDGES = 800000
IN_CH = 128
HC = 64
NEG_SLOPE = 0.2
N_CORES = 8
W_WIN = 64  # nodes per scatter window (one-hot width)


def _cfg(n_nodes=N_NODES, n_edges=N_EDGES, n_cores=N_CORES, w=W_WIN):
    nw = math.ceil(n_nodes / w)
    npc = math.ceil(nw / n_cores)  # window slots per core
    if npc % 2 == 1:
        npc += 1  # flush in pairs
    n_pad = n_cores * npc * w
    nshard = npc * w
    gw = 1
    for cand in (7, 6, 5, 4, 8, 3, 2):
        if npc % cand == 0:
            gw = cand
            break
    return dict(
        N=n_nodes, E=n_edges, NC=n_cores, W=w, NPC=npc,
        N_PAD=n_pad, NSHARD=nshard, GW=gw, NGROUPS=npc // gw,
        NPAIR=npc // 2, T_TILES=n_pad // 128, SHARD_TILES=nshard // 128,
        BASE1=n_pad - 32768, HI_LIM=32768,
    )


def _prep(inputs, cfg):
    """Host-side preprocessing: permutation, binning, index streams."""
    N, E, NC, W = cfg["N"], cfg["E"], cfg["NC"], cfg["W"]
    NPC, N_PAD, NSHARD = cfg["NPC"], cfg["N_PAD"], cfg["NSHARD"]
    GW, NGROUPS, NPAIR = cfg["GW"], cfg["NGROUPS"], cfg["NPAIR"]
    BASE1, HI_LIM = cfg["BASE1"], cfg["HI_LIM"]
    NW = N_PAD // W

    nodes_ft = np.asarray(inputs["nodes_ft"], dtype=np.float32)
    adj = np.asarray(inputs["adj_list"])
    weight = np.asarray(inputs["weight"], dtype=np.float32)
    a1 = np.asarray(inputs["att_layer_1"], dtype=np.float32)
    a2 = np.asarray(inputs["att_layer_2"], dtype=np.float32)
    bias = np.asarray(inputs["bias"], dtype=np.float32)

    tgt0 = adj[0].astype(np.int64)
    src0 = adj[1].astype(np.int64)

    # ---- window dealing permutation
    cnt_old = np.bincount(tgt0 // W, minlength=NW)
    order_w = np.argsort(-cnt_old, kind="stable")
    new_of_oldwin = np.empty(NW, dtype=np.int64)
    idxs = np.arange(NW)
    new_of_oldwin[order_w] = (idxs % NC) * NPC + idxs // NC
    perm = new_of_oldwin[np.arange(N_PAD) // W] * W + np.arange(N_PAD) % W

    tgt = perm[tgt0]
    src = perm[src0]

    # ---- per-edge core/slot/class
    c_e = tgt // NSHARD
    s_e = (tgt // W) % NPC
    tloc = (tgt % W).astype(np.float32)
    src_rot = (src - c_e * NSHARD) % N_PAD
    f_lo = src_rot < BASE1
    f_hi = src_rot >= HI_LIM
    f_fr = ~f_lo & ~f_hi

    key = c_e * NPC + s_e
    cnt_cs = np.bincount(key, minlength=NC * NPC).reshape(NC, NPC)
    flo_cs = np.bincount(key[f_lo], minlength=NC * NPC).reshape(NC, NPC)
    fhi_cs = np.bincount(key[f_hi], minlength=NC * NPC).reshape(NC, NPC)

    maxcnt = cnt_cs.max(axis=0)
    maxflo = flo_cs.max(axis=0)
    maxfhi = fhi_cs.max(axis=0)
    assert maxcnt.min() > 0, "empty slot: need dummy handling"

    lob = np.ceil(maxflo / 128).astype(np.int64)
    k_s = np.maximum(np.ceil(maxcnt / 128).astype(np.int64),
                     lob + np.ceil(maxfhi / 128).astype(np.int64))
    hib = k_s - lob
    assert (flo_cs <= lob[None, :] * 128).all()
    assert (fhi_cs <= hib[None, :] * 128).all()

    # per-(core,slot) lo occupancy
    nfree = cnt_cs - flo_cs - fhi_cs
    lo_w = np.maximum(np.minimum(flo_cs + nfree, lob[None, :] * 128),
                      cnt_cs - hib[None, :] * 128)
    quota = lo_w - flo_cs  # free edges sent to lo

    # free-edge rank within (c, s)
    fr_idx = np.flatnonzero(f_fr)
    fr_ord = fr_idx[np.argsort(key[fr_idx], kind="stable")]
    fr_cnt = np.bincount(key[fr_idx], minlength=NC * NPC)
    fr_starts = np.zeros(NC * NPC + 1, dtype=np.int64)
    fr_starts[1:] = np.cumsum(fr_cnt)
    fr_rank = np.empty(E, dtype=np.int64)
    fr_rank[fr_ord] = np.arange(len(fr_ord)) - fr_starts[key[fr_ord]]

    is_lo = f_lo.copy()
    is_lo[f_fr] = fr_rank[f_fr] < quota.reshape(-1)[key[f_fr]]

    # ranks within (c, s, class)
    tloc_i = (tgt % W).astype(np.int64)

    def class_rank(mask, sec=None):
        mi = np.flatnonzero(mask)
        if sec is None:
            order = np.argsort(key[mi], kind="stable")
        else:
            order = np.lexsort((sec[mi], key[mi]))
        mo = mi[order]
        mc = np.bincount(key[mi], minlength=NC * NPC)
        ms = np.zeros(NC * NPC + 1, dtype=np.int64)
        ms[1:] = np.cumsum(mc)
        r = np.empty(len(mi), dtype=np.int64)
        r[order] = np.arange(len(mi)) - ms[key[mo]]
        out = np.zeros(E, dtype=np.int64)
        out[mi] = r
        return out

    # first pass: arbitrary order -> bin assignment j
    rank_lo = class_rank(is_lo)
    rank_hi = class_rank(~is_lo)
    j_tmp = np.where(is_lo, rank_lo // 128, rank_hi // 128)
    # second pass: within each bin, order by window-local tgt (staircase)
    sec = j_tmp * 64 + tloc_i
    rank_lo = class_rank(is_lo, sec)
    rank_hi = class_rank(~is_lo, sec)

    # stream bases (shared across cores)
    lo_base = np.zeros(NPC, dtype=np.int64)
    lo_base[1:] = np.cumsum(lob)[:-1]
    hi_base = np.zeros(NPC, dtype=np.int64)
    hi_base[1:] = np.cumsum(hib)[:-1]
    LOB = int(lob.sum())
    HIB = int(hib.sum())
    B = int(k_s.sum())

    # group structures
    grp_slots = [list(range(g * GW, (g + 1) * GW)) for g in range(NGROUPS)]
    nlo_g = [int(lob[s0:s0 + GW].sum()) for s0 in range(0, NPC, GW)]
    nhi_g = [int(hib[s0:s0 + GW].sum()) for s0 in range(0, NPC, GW)]
    nb_g = [a + b for a, b in zip(nlo_g, nhi_g)]
    grpoff = np.zeros(NGROUPS + 1, dtype=np.int64)
    grpoff[1:] = np.cumsum(nb_g)

    # processed-order bin id for each (slot, class, j)
    g_of_s = np.arange(NPC) // GW
    pb_lo_s = grpoff[g_of_s] + (lo_base - lo_base[g_of_s * GW])
    pb_hi_s = (grpoff[g_of_s] + np.array(nlo_g)[g_of_s]
               + (hi_base - hi_base[g_of_s * GW]))

    # per-edge stream positions
    j_e = np.where(is_lo, rank_lo // 128, rank_hi // 128)
    p_e = np.where(is_lo, rank_lo % 128, rank_hi % 128)
    pb_e = np.where(is_lo, pb_lo_s[s_e] + j_e, pb_hi_s[s_e] + j_e)

    lo_s = np.zeros((NC, LOB * 128), dtype=np.int16)
    m = is_lo
    lo_s[c_e[m], (lo_base[s_e[m]] + j_e[m]) * 128 + p_e[m]] = (
        src_rot[m].astype(np.int16))
    hi_s = np.zeros((NC, HIB * 128), dtype=np.int16)
    m = ~is_lo
    hi_s[c_e[m], (hi_base[s_e[m]] + j_e[m]) * 128 + p_e[m]] = (
        (src_rot[m] - BASE1).astype(np.int16))

    tgt_col = np.full((NC, 128, B), -1.0, dtype=np.float32)
    tgt_col[c_e, p_e, pb_e] = tloc

    # staircase thresholds: per (core, bin, n): [pads..., tloc asc]
    hist = np.zeros((NC, B, W), dtype=np.int64)
    np.add.at(hist, (c_e, pb_e, tloc_i), 1)
    cum = np.cumsum(hist, axis=2)
    thr_hi = cum.astype(np.float32)
    thr_lo = thr_hi - hist.astype(np.float32)
    thr_lo = np.ascontiguousarray(thr_lo.transpose(0, 2, 1))  # [NC, W, B]
    thr_hi = np.ascontiguousarray(thr_hi.transpose(0, 2, 1))

    def wrap16(stream2d):
        ncc, L = stream2d.shape
        w = stream2d.reshape(ncc, L // 16, 16).transpose(0, 2, 1)
        return np.ascontiguousarray(np.tile(w, (1, 8, 1)))

    lo_idx = wrap16(lo_s)
    hi_idx = wrap16(hi_s)

    # permuted, transposed, padded node features
    base = np.zeros((IN_CH, N_PAD), dtype=np.float32)
    base[:, perm[np.arange(N)]] = nodes_ft.T

    wh = weight.astype(BF16_NP)
    wi = (weight @ a1).astype(BF16_NP)
    wj = (weight @ a2).astype(BF16_NP)
    bias_full = np.tile(bias[None, :], (128, NPAIR)).astype(BF16_NP)

    in_maps = []
    for c in range(NC):
        nftT = np.ascontiguousarray(np.roll(base, -c * NSHARD, axis=1))
        in_maps.append({
            "nodes_ftT": nftT.astype(BF16_NP),
            "wh": wh, "wi": wi, "wj": wj,
            "lo_idx": lo_idx[c],
            "hi_idx": hi_idx[c],
            "tgt_col": tgt_col[c].astype(BF16_NP),
            "thr_lo": thr_lo[c],
            "thr_hi": thr_hi[c],
            "bias_bc": bias_full,
        })

    meta = dict(
        k_s=k_s.tolist(), lob=lob.tolist(), hib=hib.tolist(),
        LOB=LOB, HIB=HIB, B=B,
        nlo_g=nlo_g, nhi_g=nhi_g, nb_g=nb_g,
        NBMAX=max(nb_g), perm=perm,
    )
    return in_maps, meta


def _build_program(cfg, meta, debug_dump=False, phase_limit="full", repeat=1):
    import concourse.bacc as bacc
    import concourse.bass as bass
    import concourse.mybir as mybir
    import concourse.tile as tile
    from concourse.bass import AP

    BF16 = mybir.dt.bfloat16
    F32 = mybir.dt.float32
    I16 = mybir.dt.int16
    I32 = mybir.dt.int32
    ALU = mybir.AluOpType
    ACT = mybir.ActivationFunctionType

    NPC, N_PAD, NSHARD = cfg["NPC"], cfg["N_PAD"], cfg["NSHARD"]
    T_TILES, SHARD_TILES = cfg["T_TILES"], cfg["SHARD_TILES"]
    GW, NGROUPS, NPAIR = cfg["GW"], cfg["NGROUPS"], cfg["NPAIR"]
    BASE1 = cfg["BASE1"]
    W = cfg["W"]

    k_s, lob, hib = meta["k_s"], meta["lob"], meta["hib"]
    LOB, HIB, B = meta["LOB"], meta["HIB"], meta["B"]
    nlo_g, nhi_g, nb_g = meta["nlo_g"], meta["nhi_g"], meta["nb_g"]
    NBMAX = meta["NBMAX"]
    NLOMAX = max(nlo_g)
    NHIMAX = max(nhi_g)
    CHUNK = int(os.environ.get("GAT_CHUNK", "32"))  # bins per gather call
    GBUFS = int(os.environ.get("GAT_GBUFS", "3"))
    DEFER = bool(int(os.environ.get("GAT_DEFER", "1")))

    nc = bacc.Bacc("TRN2", target_bir_lowering=False, debug=False,
                   num_swdge_queues=4)

    nodes_ftT = nc.dram_tensor("nodes_ftT", [IN_CH, N_PAD], BF16,
                               kind="ExternalInput")
    wh_d = nc.dram_tensor("wh", [IN_CH, HC], BF16, kind="ExternalInput")
    wi_d = nc.dram_tensor("wi", [IN_CH, HC], BF16, kind="ExternalInput")
    wj_d = nc.dram_tensor("wj", [IN_CH, HC], BF16, kind="ExternalInput")
    loidx_d = nc.dram_tensor("lo_idx", [128, LOB * 8], I16,
                             kind="ExternalInput")
    hiidx_d = nc.dram_tensor("hi_idx", [128, HIB * 8], I16,
                             kind="ExternalInput")
    tgtcol_d = nc.dram_tensor("tgt_col", [128, B], BF16, kind="ExternalInput")
    thrlo_d = nc.dram_tensor("thr_lo", [W, B], F32, kind="ExternalInput")
    thrhi_d = nc.dram_tensor("thr_hi", [W, B], F32, kind="ExternalInput")
    bias_d = nc.dram_tensor("bias_bc", [128, NPAIR * HC], BF16,
                            kind="ExternalInput")
    out_d = nc.dram_tensor("out", [NSHARD, HC], F32, kind="ExternalOutput")

    hj_table = nc.dram_tensor("hj_table", [N_PAD, 2 * HC], BF16,
                              kind="Internal")

    do_build = phase_limit != "noop"
    do_gather = phase_limit in ("gather", "noatt", "nomm", "full")
    do_att = phase_limit in ("noatt", "nomm", "full")  # onehot/att builds
    do_dve = phase_limit in ("nomm", "full")
    do_mm = phase_limit == "full"

    with tile.TileContext(nc) as tc, ExitStack() as ctx:
        const_pool = ctx.enter_context(tc.tile_pool(name="const", bufs=1))
        b_in = ctx.enter_context(tc.tile_pool(name="b_in", bufs=4))
        b_ps = ctx.enter_context(tc.tile_pool(name="b_ps", bufs=2,
                                              space="PSUM"))
        b_st = ctx.enter_context(tc.tile_pool(name="b_st", bufs=4))
        idx_pool = ctx.enter_context(tc.tile_pool(name="idx", bufs=2))
        g_pool = ctx.enter_context(tc.tile_pool(name="gp", bufs=GBUFS))
        oh_pool = ctx.enter_context(tc.tile_pool(name="ohp", bufs=2))
        tr_pool = ctx.enter_context(tc.tile_pool(name="trp", bufs=2))
        sca_pool = ctx.enter_context(tc.tile_pool(name="scap", bufs=1))
        s_pool = ctx.enter_context(tc.tile_pool(name="sp", bufs=1))
        att_ps = ctx.enter_context(tc.tile_pool(name="attps", bufs=2,
                                                space="PSUM"))
        mm_ps = ctx.enter_context(tc.tile_pool(name="mmps", bufs=4,
                                               space="PSUM"))
        fl_pool = ctx.enter_context(tc.tile_pool(name="fl", bufs=1))

        wh_sb = const_pool.tile([IN_CH, HC], BF16)
        nc.sync.dma_start(wh_sb[:], wh_d[:])
        wi_sb = const_pool.tile([IN_CH, HC], BF16)
        nc.sync.dma_start(wi_sb[:], wi_d[:])
        wj_sb = const_pool.tile([IN_CH, HC], BF16)
        nc.sync.dma_start(wj_sb[:], wj_d[:])
        bias_sb = const_pool.tile([128, NPAIR * HC], BF16)
        nc.sync.dma_start(bias_sb[:], bias_d[:])

        # iota constants
        iota_ri = const_pool.tile([128, W], I32)
        nc.gpsimd.iota(iota_ri[:], pattern=[[1, W]], base=0,
                       channel_multiplier=0)
        iota_row = const_pool.tile([128, W], BF16)
        nc.vector.tensor_copy(iota_row[:], iota_ri[:])
        iota_f64i = const_pool.tile([64, 128], I32)
        nc.gpsimd.iota(iota_f64i[:], pattern=[[1, 128]], base=0,
                       channel_multiplier=0)
        iota_f64 = const_pool.tile([64, 128], F32)
        nc.vector.tensor_copy(iota_f64[:], iota_f64i[:])

        # att_i table for this core's shard, [64, NPC, HC]
        aw_sb = const_pool.tile([64, NPC, HC], BF16)

        def emit_once(rep):
            # ---- Phase A: hj table (h | att_j), batched 2 tiles / iter
            for t2 in range(T_TILES // 2 if do_build else 0):
                t = 2 * t2
                dmae = nc.sync if t2 % 2 == 0 else nc.scalar
                nf = b_in.tile([128, 2, 128], BF16, name="nf")
                dmae.dma_start(
                    nf[:].rearrange("p a b -> p (a b)"),
                    nodes_ftT[:, 128 * t:128 * (t + 2)])
                ps = b_ps.tile([128, 2, 2 * HC], F32, name="bps")
                for u in range(2):
                    nc.tensor.matmul(ps[:, u, 0:HC], nf[:, u, :], wh_sb[:],
                                     start=(u == 0), stop=False)
                    nc.tensor.matmul(ps[:, u, HC:2 * HC], nf[:, u, :],
                                     wj_sb[:], start=False, stop=(u == 1))
                st = b_st.tile([128, 2, 2 * HC], BF16, name="bst")
                if t2 % 2 == 0:
                    nc.vector.tensor_copy(st[:], ps[:])
                else:
                    nc.scalar.copy(st[:], ps[:])
                dmae.dma_start(
                    hj_table[128 * t:128 * (t + 2), :].rearrange(
                        "(a p) b -> p a b", p=128),
                    st[:])
            # att_i for own shard -> SBUF aw (both 64-col halves to part 0-63)
            for t in range(SHARD_TILES if do_build else 0):
                nf2 = b_in.tile([128, 128], BF16, tag="nf2", name="nf2")
                dmae = nc.scalar if t % 2 == 0 else nc.sync
                dmae.dma_start(nf2[:], nodes_ftT[:, 128 * t:128 * (t + 1)])
                ps2f = b_ps.tile([128, 2, 2 * HC], F32, tag="bps",
                                 name="bps2")
                ps2 = ps2f[0:64, :, 0:HC]
                nc.tensor.matmul(ps2[:, 0, :], nf2[:, 0:64], wi_sb[:],
                                 start=True, stop=True)
                nc.tensor.matmul(ps2[:, 1, :], nf2[:, 64:128], wi_sb[:],
                                 start=True, stop=True)
                if t % 2 == 0:
                    nc.scalar.copy(aw_sb[:, 2 * t:2 * t + 2, :], ps2[:])
                else:
                    nc.vector.tensor_copy(aw_sb[:, 2 * t:2 * t + 2, :],
                                          ps2[:])

            if int(os.environ.get("GAT_BARRIER", "0")):
                tc.strict_bb_all_engine_barrier()

            # ---- Phase B
            stage_n = fl_pool.tile([128, NPAIR * HC], F32, tag="sn",
                                   name="sn")
            stage_d = fl_pool.tile([128, NPAIR * HC], F32, tag="sd",
                                   name="sd")

            pair_tiles = {}
            done_slots = set()
            qn = 0
            lo_off = 0
            hi_off = 0
            boff = 0
            last_G = last_OH = last_TR = None
            prev_grp = None

            def emit_scatter(bins_p, OH_p, G_p):
                for bl in range(len(bins_p)):
                    s_bl, jj = bins_p[bl]
                    pr, half = s_bl // 2, s_bl % 2
                    if pr not in pair_tiles:
                        pair_tiles[pr] = mm_ps.tile(
                            [128, 2 * HC], F32, tag="pp", name=f"pp{pr}")
                    ps_t = pair_tiles[pr]
                    nc.tensor.matmul(
                        ps_t[HC * half:HC * half + HC, :],
                        OH_p[:, bl, :],
                        G_p[:, bl, :],
                        start=(jj == 0), stop=(jj == k_s[s_bl] - 1),
                        tile_position=(0, HC * half),
                        skip_group_check=True,
                    )
                    if jj == k_s[s_bl] - 1:
                        done_slots.add(s_bl)
                        if (s_bl ^ 1) in done_slots:
                            flush_pair(pr)

            def flush_pair(pr):
                ps_t = pair_tiles.pop(pr)
                nc.vector.tensor_copy(
                    stage_n[:, HC * pr:HC * (pr + 1)], ps_t[:, 0:HC])
                nc.vector.tensor_copy(
                    stage_d[:, HC * pr:HC * (pr + 1)], ps_t[:, HC:2 * HC])

            for g in range(NGROUPS if do_gather else 0):
                slots = list(range(g * GW, (g + 1) * GW))
                nlo, nhi, NB = nlo_g[g], nhi_g[g], nb_g[g]
                # bins in processed order: (slot, jj) jj in 0..k_s-1
                bins = []
                for s in slots:
                    bins += [(s, j) for j in range(lob[s])]
                for s in slots:
                    bins += [(s, lob[s] + j) for j in range(hib[s])]

                sl_t = idx_pool.tile([128, NLOMAX * 8], I16, tag="sl",
                                     name="sl")
                sl = sl_t[:, 0:nlo * 8]
                nc.sync.dma_start(sl,
                                  loidx_d[:, lo_off * 8:(lo_off + nlo) * 8])
                sh_t = idx_pool.tile([128, NHIMAX * 8], I16, tag="sh",
                                     name="sh")
                sh = sh_t[:, 0:nhi * 8]
                nc.sync.dma_start(sh,
                                  hiidx_d[:, hi_off * 8:(hi_off + nhi) * 8])
                tct_t = idx_pool.tile([128, NBMAX], BF16, tag="tc", name="tc")
                nc.scalar.dma_start(tct_t[:, 0:NB], tgtcol_d[:, boff:boff + NB])
                thl_t = idx_pool.tile([W, NBMAX], F32, tag="thl", name="thl")
                nc.scalar.dma_start(thl_t[:, 0:NB], thrlo_d[:, boff:boff + NB])
                thh_t = idx_pool.tile([W, NBMAX], F32, tag="thh", name="thh")
                nc.scalar.dma_start(thh_t[:, 0:NB], thrhi_d[:, boff:boff + NB])

                G_t = g_pool.tile([128, NBMAX, 2 * HC], BF16, tag="G",
                                  name="G")
                G = G_t[:, 0:NB, :]

                def chunked_gather(obase, table_ap, idx_tile, nbins):
                    nonlocal qn
                    parts = max(1, (nbins + CHUNK - 1) // CHUNK)
                    cuts = [nbins * i // parts for i in range(parts + 1)]
                    for a, b2 in zip(cuts[:-1], cuts[1:]):
                        if a == b2:
                            continue
                        nc.gpsimd.dma_gather(
                            out_ap=G_t[:, obase + a:obase + b2, :],
                            in_ap=table_ap,
                            idxs_ap=idx_tile[:, a * 8:b2 * 8],
                            num_idxs=(b2 - a) * 128,
                            num_idxs_reg=(b2 - a) * 128,
                            elem_size=2 * HC, queue_num=qn % 4,
                            single_packet=False,
                        )
                        qn += 1

                chunked_gather(0, hj_table[0:32768, :], sl, nlo)
                chunked_gather(nlo, hj_table[BASE1:N_PAD, :], sh, nhi)
                last_G = G

                if not do_att:
                    lo_off += nlo
                    hi_off += nhi
                    boff += NB
                    continue

                # onehot [128, NB, 64]
                OH_t = oh_pool.tile([128, NBMAX, W], BF16, tag="OH",
                                    name="OH")
                OH = OH_t[:, 0:NB, :]
                t0 = tct_t[:, 0:NB]
                in0 = AP(t0.tensor, t0.offset,
                         [list(t0.ap[0]), list(t0.ap[1]), [0, W]])
                i1 = iota_row[:]
                in1 = AP(i1.tensor, i1.offset,
                         [list(i1.ap[0]), [0, NB], list(i1.ap[1])])
                nc.vector.tensor_tensor(out=OH[:], in0=in0, in1=in1,
                                        op=ALU.is_equal)

                # onehotT [64, NB, 128] staircase: bins are tloc-sorted,
                # so row n of a bin is 1 on [thr_lo[n], thr_hi[n])
                TR_t = tr_pool.tile([64, NBMAX, 128], BF16, tag="TR",
                                    name="TR")
                TR = TR_t[:, 0:NB, :]
                last_OH, last_TR = OH, TR
                a_full = sca_pool.tile([64, NBMAX, 128], BF16, tag="sca",
                                       name="sca")
                af = a_full[:, 0:NB, :]
                io = iota_f64[:]
                io_b = AP(io.tensor, io.offset,
                          [list(io.ap[0]), [0, NB], list(io.ap[1])])
                tl_ap = thl_t[:, 0:NB]
                tl_b = AP(tl_ap.tensor, tl_ap.offset,
                          [list(tl_ap.ap[0]), list(tl_ap.ap[1]), [0, 128]])
                th_ap = thh_t[:, 0:NB]
                th_b = AP(th_ap.tensor, th_ap.offset,
                          [list(th_ap.ap[0]), list(th_ap.ap[1]), [0, 128]])
                if do_att:
                    nc.vector.tensor_tensor(out=af, in0=io_b, in1=tl_b,
                                            op=ALU.is_ge)
                    nc.vector.tensor_tensor(out=TR, in0=io_b, in1=th_b,
                                            op=ALU.is_lt)
                    nc.vector.tensor_tensor(out=TR, in0=TR, in1=af,
                                            op=ALU.mult)

                # att_i matmuls + s = att_j + att_i, 8 bins per psum bank
                S_t = s_pool.tile([128, NBMAX, HC], BF16, tag="S", name="S")
                S = S_t[:, 0:NB, :]
                if do_dve:
                    for c0 in range(0, NB, 8):
                        c1 = min(c0 + 8, NB)
                        ap_t = att_ps.tile([128, 8, HC], F32, tag="att",
                                           name="att")
                        for bl in range(c0, c1):
                            s_bl = bins[bl][0]
                            nc.tensor.matmul(
                                ap_t[:, bl - c0, :], TR[:, bl, :],
                                aw_sb[:, s_bl, :], start=True, stop=True)
                        nc.vector.tensor_tensor(
                            out=S[:, c0:c1, :], in0=G[:, c0:c1, HC:2 * HC],
                            in1=ap_t[:, 0:c1 - c0, :], op=ALU.add)
                    # leaky relu; x = exp; y = x * h
                    nc.vector.scalar_tensor_tensor(
                        out=S, in0=S, scalar=NEG_SLOPE, in1=S,
                        op0=ALU.mult, op1=ALU.max)
                    nc.scalar.activation(G[:, :, HC:2 * HC], S, ACT.Exp)
                    nc.vector.tensor_tensor(
                        out=G[:, :, 0:HC], in0=G[:, :, 0:HC],
                        in1=G[:, :, HC:2 * HC], op=ALU.mult)

                if do_mm:
                    if DEFER:
                        if prev_grp is not None:
                            emit_scatter(*prev_grp)
                        prev_grp = (bins, OH, G)
                    else:
                        emit_scatter(bins, OH, G)

                lo_off += nlo
                hi_off += nhi
                boff += NB

            if do_mm and prev_grp is not None:
                emit_scatter(*prev_grp)

            # ---- Phase C
            if not do_mm:
                nc.vector.memset(stage_n[:], 0.0)
                nc.vector.memset(stage_d[:], 1.0)
            assert not pair_tiles, f"unflushed pairs: {list(pair_tiles)}"
            nc.vector.tensor_scalar_add(stage_d[:], stage_d[:], 1e-16)
            nc.scalar.activation(stage_d[:], stage_d[:], ACT.Ln)
            nc.scalar.activation(stage_d[:], stage_d[:], ACT.Exp, scale=-1.0)
            nc.vector.tensor_tensor(out=stage_n[:], in0=stage_n[:],
                                    in1=stage_d[:], op=ALU.mult)
            nc.vector.tensor_tensor(out=stage_n[:], in0=stage_n[:],
                                    in1=bias_sb[:], op=ALU.add)

            out_view = out_d[:].rearrange("(pr p) c -> p pr c", p=128)
            st_view = stage_n[:].rearrange("p (pr c) -> p pr c", c=HC)
            nc.sync.dma_start(out_view, st_view)
            return last_G, last_OH, last_TR, stage_d

        for rep in range(repeat):
            outs = emit_once(rep)
            if repeat > 1:
                tc.strict_bb_all_engine_barrier()
        last_G, last_OH, last_TR, stage_d = outs

        if debug_dump:
            NB_l = nb_g[-1]
            dump_hj = nc.dram_tensor("dump_hj", [N_PAD, 2 * HC], BF16,
                                     kind="ExternalOutput")
            dump_aw = nc.dram_tensor("dump_aw", [64, NPC * HC], BF16,
                                     kind="ExternalOutput")
            dump_sd = nc.dram_tensor("dump_sd", [128, NPAIR * HC], F32,
                                     kind="ExternalOutput")
            dump_g = nc.dram_tensor("dump_g", [128, NB_l * 2 * HC], BF16,
                                    kind="ExternalOutput")
            dump_oh = nc.dram_tensor("dump_oh", [128, NB_l * W], BF16,
                                     kind="ExternalOutput")
            dump_tr = nc.dram_tensor("dump_tr", [64, NB_l * 128], BF16,
                                     kind="ExternalOutput")
            tc.strict_bb_all_engine_barrier()
            nc.sync.dma_start(dump_hj[:], hj_table[:])
            nc.sync.dma_start(dump_aw[:],
                              aw_sb[:].rearrange("p a b -> p (a b)"))
            nc.sync.dma_start(dump_sd[:], stage_d[:])
            nc.sync.dma_start(dump_g[:],
                              last_G[:].rearrange("p a b -> p (a b)"))
            nc.sync.dma_start(dump_oh[:],
                              last_OH[:].rearrange("p a b -> p (a b)"))
            nc.sync.dma_start(dump_tr[:],
                              last_TR[:].rearrange("p a b -> p (a b)"))

    nc.compile()
    return nc


def kernel(**inputs):
    cfg = _cfg()
    in_maps, meta = _prep(inputs, cfg)
    nc = _build_program(cfg, meta)

    from concourse import bass_utils
    res = bass_utils.run_bass_kernel_spmd(
        nc, in_maps, core_ids=list(range(cfg["NC"])),
        trace=bool(int(os.environ.get("GAT_TRACE", "0"))),
    )
    kernel.last_result = res
    kernel.last_ctx = (nc, in_maps, cfg, meta)

    NSHARD = cfg["NSHARD"]
    out_new = np.concatenate(
        [res.results[c]["out"] for c in range(cfg["NC"])], axis=0)
    perm = meta["perm"]
    return out_new[perm[:cfg["N"]]]


# revision 21
# speedup vs baseline: 1.3302x; 1.1706x over previous
"""Bass/Trainium2 kernel for nn_BitGatConv (GAT-style message passing).

Self-contained: takes full inputs, shards edges by destination window across
8 NeuronCores (SPMD, one program), returns the full [N, HC] output.

v2 design (per core, window-dealt + rotated node ids so all cores run the
same program):
  Host prep: windows (64 dst nodes each) are sorted by edge count and dealt
    round-robin to (slot, core) so every core's slot s has a similar count;
    the node ids are permuted accordingly (output unpermuted at the end).
    Edges are binned per (core, slot) into k_s bins of 128 edge slots
    (k_s shared across cores = max need). Each bin's edges gather from one
    of two overlapping table halves (rows 0..32767 / rows 17408..50175) so
    int16 gather indices reach the whole 50176-row table; edges whose src
    falls in the 15360-row overlap are assigned to whichever half aligns
    the per-slot split to a bin boundary (minimal padding).
  Phase A (build): h = nodes_ft @ W, att_j = nodes_ft @ (W@A2) stored to a
    DRAM table hj_table [N_PAD, 128] rows = [h | att_j] (bf16);
    att_i = nodes_ft @ (W@A1) for the core's own shard kept in SBUF
    (aw [64, NPC, 64]).
  Phase B (edges), per group of GW slots:
    - dma_gather hj rows by src (the ONLY per-edge HBM gather: 256B/edge)
    - build onehot [128e, nb, 64n] on-chip: is_equal(tgt bcast, iota row)
    - build onehotT [64n, nb*128e]: gpsimd partition_broadcast of the
      bin-major tgt stream, then is_equal vs per-partition iota column
    - att_i per edge via PE: psum[e, ch] = onehotT^T @ aw_slot
    - s = att_j + att_i; l = max(0.2 s, s); x = exp(l) (shift-free softmax:
      logits are bounded so exp is safe); y = x * h
    - scatter via PE: pair psum [128, 128] accumulates onehot^T-weighted
      [y | x] per window over the slot's k_s bins (v1-proven tile_position
      packing of two windows per psum tile)
  Phase C (flush): out = numer / (denom + 1e-16) + bias -> [NSHARD, HC];
    host inverts the window permutation.
"""

import math
import os
import sys
from contextlib import ExitStack

import numpy as np

for _p in ("/opt/trn_rl_repo",):
    if _p not in sys.path:
        sys.path.insert(0, _p)

import ml_dtypes  # noqa: E402

BF16_NP = ml_dtypes.bfloat16

# ---------------------------------------------------------------------------
# Problem constants (hardcoded per contest rules)
N_NODES = 50000
N_EDGES = 800000
IN_CH = 128
HC = 64
NEG_SLOPE = 0.2
N_CORES = 8
W_WIN = 64  # nodes per scatter window (one-hot width)


def _cfg(n_nodes=N_NODES, n_edges=N_EDGES, n_cores=N_CORES, w=W_WIN):
    nw = math.ceil(n_nodes / w)
    npc = math.ceil(nw / n_cores)  # window slots per core
    if npc % 2 == 1:
        npc += 1  # flush in pairs
    n_pad = n_cores * npc * w
    nshard = npc * w
    gw = 1
    for cand in (7, 6, 5, 4, 8, 3, 2):
        if npc % cand == 0:
            gw = cand
            break
    return dict(
        N=n_nodes, E=n_edges, NC=n_cores, W=w, NPC=npc,
        N_PAD=n_pad, NSHARD=nshard, GW=gw, NGROUPS=npc // gw,
        NPAIR=npc // 2, T_TILES=n_pad // 128, SHARD_TILES=nshard // 128,
        BASE1=n_pad - 32768, HI_LIM=32768,
    )


def _prep(inputs, cfg):
    """Host-side preprocessing: permutation, binning, index streams."""
    N, E, NC, W = cfg["N"], cfg["E"], cfg["NC"], cfg["W"]
    NPC, N_PAD, NSHARD = cfg["NPC"], cfg["N_PAD"], cfg["NSHARD"]
    GW, NGROUPS, NPAIR = cfg["GW"], cfg["NGROUPS"], cfg["NPAIR"]
    BASE1, HI_LIM = cfg["BASE1"], cfg["HI_LIM"]
    NW = N_PAD // W

    nodes_ft = np.asarray(inputs["nodes_ft"], dtype=np.float32)
    adj = np.asarray(inputs["adj_list"])
    weight = np.asarray(inputs["weight"], dtype=np.float32)
    a1 = np.asarray(inputs["att_layer_1"], dtype=np.float32)
    a2 = np.asarray(inputs["att_layer_2"], dtype=np.float32)
    bias = np.asarray(inputs["bias"], dtype=np.float32)

    tgt0 = adj[0].astype(np.int64)
    src0 = adj[1].astype(np.int64)

    # ---- window dealing permutation
    cnt_old = np.bincount(tgt0 // W, minlength=NW)
    order_w = np.argsort(-cnt_old, kind="stable")
    new_of_oldwin = np.empty(NW, dtype=np.int64)
    idxs = np.arange(NW)
    new_of_oldwin[order_w] = (idxs % NC) * NPC + idxs // NC
    perm = new_of_oldwin[np.arange(N_PAD) // W] * W + np.arange(N_PAD) % W

    tgt = perm[tgt0]
    src = perm[src0]

    # ---- per-edge core/slot/class
    c_e = tgt // NSHARD
    s_e = (tgt // W) % NPC
    tloc = (tgt % W).astype(np.float32)
    src_rot = (src - c_e * NSHARD) % N_PAD
    f_lo = src_rot < BASE1
    f_hi = src_rot >= HI_LIM
    f_fr = ~f_lo & ~f_hi

    key = c_e * NPC + s_e
    cnt_cs = np.bincount(key, minlength=NC * NPC).reshape(NC, NPC)
    flo_cs = np.bincount(key[f_lo], minlength=NC * NPC).reshape(NC, NPC)
    fhi_cs = np.bincount(key[f_hi], minlength=NC * NPC).reshape(NC, NPC)

    maxcnt = cnt_cs.max(axis=0)
    maxflo = flo_cs.max(axis=0)
    maxfhi = fhi_cs.max(axis=0)
    assert maxcnt.min() > 0, "empty slot: need dummy handling"

    lob = np.ceil(maxflo / 128).astype(np.int64)
    k_s = np.maximum(np.ceil(maxcnt / 128).astype(np.int64),
                     lob + np.ceil(maxfhi / 128).astype(np.int64))
    hib = k_s - lob
    assert (flo_cs <= lob[None, :] * 128).all()
    assert (fhi_cs <= hib[None, :] * 128).all()

    # per-(core,slot) lo occupancy
    nfree = cnt_cs - flo_cs - fhi_cs
    lo_w = np.maximum(np.minimum(flo_cs + nfree, lob[None, :] * 128),
                      cnt_cs - hib[None, :] * 128)
    quota = lo_w - flo_cs  # free edges sent to lo

    # free-edge rank within (c, s)
    fr_idx = np.flatnonzero(f_fr)
    fr_ord = fr_idx[np.argsort(key[fr_idx], kind="stable")]
    fr_cnt = np.bincount(key[fr_idx], minlength=NC * NPC)
    fr_starts = np.zeros(NC * NPC + 1, dtype=np.int64)
    fr_starts[1:] = np.cumsum(fr_cnt)
    fr_rank = np.empty(E, dtype=np.int64)
    fr_rank[fr_ord] = np.arange(len(fr_ord)) - fr_starts[key[fr_ord]]

    is_lo = f_lo.copy()
    is_lo[f_fr] = fr_rank[f_fr] < quota.reshape(-1)[key[f_fr]]

    # ranks within (c, s, class)
    tloc_i = (tgt % W).astype(np.int64)

    def class_rank(mask, sec=None):
        mi = np.flatnonzero(mask)
        if sec is None:
            order = np.argsort(key[mi], kind="stable")
        else:
            order = np.lexsort((sec[mi], key[mi]))
        mo = mi[order]
        mc = np.bincount(key[mi], minlength=NC * NPC)
        ms = np.zeros(NC * NPC + 1, dtype=np.int64)
        ms[1:] = np.cumsum(mc)
        r = np.empty(len(mi), dtype=np.int64)
        r[order] = np.arange(len(mi)) - ms[key[mo]]
        out = np.zeros(E, dtype=np.int64)
        out[mi] = r
        return out

    # first pass: arbitrary order -> bin assignment j
    rank_lo = class_rank(is_lo)
    rank_hi = class_rank(~is_lo)
    j_tmp = np.where(is_lo, rank_lo // 128, rank_hi // 128)
    # second pass: within each bin, order by window-local tgt (staircase)
    sec = j_tmp * 64 + tloc_i
    rank_lo = class_rank(is_lo, sec)
    rank_hi = class_rank(~is_lo, sec)

    # stream bases (shared across cores)
    lo_base = np.zeros(NPC, dtype=np.int64)
    lo_base[1:] = np.cumsum(lob)[:-1]
    hi_base = np.zeros(NPC, dtype=np.int64)
    hi_base[1:] = np.cumsum(hib)[:-1]
    LOB = int(lob.sum())
    HIB = int(hib.sum())
    B = int(k_s.sum())

    # group structures
    grp_slots = [list(range(g * GW, (g + 1) * GW)) for g in range(NGROUPS)]
    nlo_g = [int(lob[s0:s0 + GW].sum()) for s0 in range(0, NPC, GW)]
    nhi_g = [int(hib[s0:s0 + GW].sum()) for s0 in range(0, NPC, GW)]
    nb_g = [a + b for a, b in zip(nlo_g, nhi_g)]
    grpoff = np.zeros(NGROUPS + 1, dtype=np.int64)
    grpoff[1:] = np.cumsum(nb_g)

    # processed-order bin id for each (slot, class, j)
    g_of_s = np.arange(NPC) // GW
    pb_lo_s = grpoff[g_of_s] + (lo_base - lo_base[g_of_s * GW])
    pb_hi_s = (grpoff[g_of_s] + np.array(nlo_g)[g_of_s]
               + (hi_base - hi_base[g_of_s * GW]))

    # per-edge stream positions
    j_e = np.where(is_lo, rank_lo // 128, rank_hi // 128)
    p_e = np.where(is_lo, rank_lo % 128, rank_hi % 128)
    pb_e = np.where(is_lo, pb_lo_s[s_e] + j_e, pb_hi_s[s_e] + j_e)

    lo_s = np.zeros((NC, LOB * 128), dtype=np.int16)
    m = is_lo
    lo_s[c_e[m], (lo_base[s_e[m]] + j_e[m]) * 128 + p_e[m]] = (
        src_rot[m].astype(np.int16))
    hi_s = np.zeros((NC, HIB * 128), dtype=np.int16)
    m = ~is_lo
    hi_s[c_e[m], (hi_base[s_e[m]] + j_e[m]) * 128 + p_e[m]] = (
        (src_rot[m] - BASE1).astype(np.int16))

    tgt_col = np.full((NC, 128, B), -1.0, dtype=np.float32)
    tgt_col[c_e, p_e, pb_e] = tloc

    # staircase thresholds: per (core, bin, n): [pads..., tloc asc]
    hist = np.zeros((NC, B, W), dtype=np.int64)
    np.add.at(hist, (c_e, pb_e, tloc_i), 1)
    cum = np.cumsum(hist, axis=2)
    thr_hi = cum.astype(np.float32)
    thr_lo = thr_hi - hist.astype(np.float32)
    thr_lo = np.ascontiguousarray(thr_lo.transpose(0, 2, 1))  # [NC, W, B]
    thr_hi = np.ascontiguousarray(thr_hi.transpose(0, 2, 1))

    def wrap16(stream2d):
        ncc, L = stream2d.shape
        w = stream2d.reshape(ncc, L // 16, 16).transpose(0, 2, 1)
        return np.ascontiguousarray(np.tile(w, (1, 8, 1)))

    lo_idx = wrap16(lo_s)
    hi_idx = wrap16(hi_s)

    # permuted, transposed, padded node features
    base = np.zeros((IN_CH, N_PAD), dtype=np.float32)
    base[:, perm[np.arange(N)]] = nodes_ft.T

    wh = weight.astype(BF16_NP)
    wi = (weight @ a1).astype(BF16_NP)
    wj = (weight @ a2).astype(BF16_NP)
    bias_full = np.tile(bias[None, :], (128, NPAIR)).astype(BF16_NP)

    in_maps = []
    for c in range(NC):
        nftT = np.ascontiguousarray(np.roll(base, -c * NSHARD, axis=1))
        in_maps.append({
            "nodes_ftT": nftT.astype(BF16_NP),
            "wh": wh, "wi": wi, "wj": wj,
            "lo_idx": lo_idx[c],
            "hi_idx": hi_idx[c],
            "tgt_col": tgt_col[c].astype(BF16_NP),
            "thr_lo": thr_lo[c],
            "thr_hi": thr_hi[c],
            "bias_bc": bias_full,
        })

    meta = dict(
        k_s=k_s.tolist(), lob=lob.tolist(), hib=hib.tolist(),
        LOB=LOB, HIB=HIB, B=B,
        nlo_g=nlo_g, nhi_g=nhi_g, nb_g=nb_g,
        NBMAX=max(nb_g), perm=perm,
    )
    return in_maps, meta


def _build_program(cfg, meta, debug_dump=False, phase_limit="full", repeat=1):
    import concourse.bacc as bacc
    import concourse.bass as bass
    import concourse.mybir as mybir
    import concourse.tile as tile
    from concourse.bass import AP

    BF16 = mybir.dt.bfloat16
    F32 = mybir.dt.float32
    I16 = mybir.dt.int16
    I32 = mybir.dt.int32
    ALU = mybir.AluOpType
    ACT = mybir.ActivationFunctionType

    NPC, N_PAD, NSHARD = cfg["NPC"], cfg["N_PAD"], cfg["NSHARD"]
    T_TILES, SHARD_TILES = cfg["T_TILES"], cfg["SHARD_TILES"]
    GW, NGROUPS, NPAIR = cfg["GW"], cfg["NGROUPS"], cfg["NPAIR"]
    BASE1 = cfg["BASE1"]
    W = cfg["W"]

    k_s, lob, hib = meta["k_s"], meta["lob"], meta["hib"]
    LOB, HIB, B = meta["LOB"], meta["HIB"], meta["B"]
    nlo_g, nhi_g, nb_g = meta["nlo_g"], meta["nhi_g"], meta["nb_g"]
    NBMAX = meta["NBMAX"]
    NLOMAX = max(nlo_g)
    NHIMAX = max(nhi_g)
    CHUNK = int(os.environ.get("GAT_CHUNK", "6"))  # bins per gather call
    GBUFS = int(os.environ.get("GAT_GBUFS", "3"))
    DEFER = bool(int(os.environ.get("GAT_DEFER", "1")))

    nc = bacc.Bacc("TRN2", target_bir_lowering=False, debug=False,
                   num_swdge_queues=4)

    nodes_ftT = nc.dram_tensor("nodes_ftT", [IN_CH, N_PAD], BF16,
                               kind="ExternalInput")
    wh_d = nc.dram_tensor("wh", [IN_CH, HC], BF16, kind="ExternalInput")
    wi_d = nc.dram_tensor("wi", [IN_CH, HC], BF16, kind="ExternalInput")
    wj_d = nc.dram_tensor("wj", [IN_CH, HC], BF16, kind="ExternalInput")
    loidx_d = nc.dram_tensor("lo_idx", [128, LOB * 8], I16,
                             kind="ExternalInput")
    hiidx_d = nc.dram_tensor("hi_idx", [128, HIB * 8], I16,
                             kind="ExternalInput")
    tgtcol_d = nc.dram_tensor("tgt_col", [128, B], BF16, kind="ExternalInput")
    thrlo_d = nc.dram_tensor("thr_lo", [W, B], F32, kind="ExternalInput")
    thrhi_d = nc.dram_tensor("thr_hi", [W, B], F32, kind="ExternalInput")
    bias_d = nc.dram_tensor("bias_bc", [128, NPAIR * HC], BF16,
                            kind="ExternalInput")
    out_d = nc.dram_tensor("out", [NSHARD, HC], F32, kind="ExternalOutput")

    hj_table = nc.dram_tensor("hj_table", [N_PAD, 2 * HC], BF16,
                              kind="Internal")

    do_build = phase_limit != "noop"
    do_gather = phase_limit in ("gather", "noatt", "nomm", "full")
    do_att = phase_limit in ("noatt", "nomm", "full")  # onehot/att builds
    do_dve = phase_limit in ("nomm", "full")
    do_mm = phase_limit == "full"

    with tile.TileContext(nc) as tc, ExitStack() as ctx:
        const_pool = ctx.enter_context(tc.tile_pool(name="const", bufs=1))
        b_in = ctx.enter_context(tc.tile_pool(name="b_in", bufs=4))
        b_ps = ctx.enter_context(tc.tile_pool(name="b_ps", bufs=2,
                                              space="PSUM"))
        b_st = ctx.enter_context(tc.tile_pool(name="b_st", bufs=4))
        idx_pool = ctx.enter_context(tc.tile_pool(name="idx", bufs=2))
        g_pool = ctx.enter_context(tc.tile_pool(name="gp", bufs=GBUFS))
        oh_pool = ctx.enter_context(tc.tile_pool(name="ohp", bufs=3))
        tr_pool = ctx.enter_context(tc.tile_pool(name="trp", bufs=2))
        sca_pool = ctx.enter_context(tc.tile_pool(name="scap", bufs=1))
        s_pool = ctx.enter_context(tc.tile_pool(name="sp", bufs=1))
        att_ps = ctx.enter_context(tc.tile_pool(name="attps", bufs=2,
                                                space="PSUM"))
        mm_ps = ctx.enter_context(tc.tile_pool(name="mmps", bufs=4,
                                               space="PSUM"))
        fl_pool = ctx.enter_context(tc.tile_pool(name="fl", bufs=1))

        wh_sb = const_pool.tile([IN_CH, HC], BF16)
        nc.sync.dma_start(wh_sb[:], wh_d[:])
        wi_sb = const_pool.tile([IN_CH, HC], BF16)
        nc.sync.dma_start(wi_sb[:], wi_d[:])
        wj_sb = const_pool.tile([IN_CH, HC], BF16)
        nc.sync.dma_start(wj_sb[:], wj_d[:])
        bias_sb = const_pool.tile([128, NPAIR * HC], BF16)
        nc.sync.dma_start(bias_sb[:], bias_d[:])

        # iota constants
        iota_ri = const_pool.tile([128, W], I32)
        nc.gpsimd.iota(iota_ri[:], pattern=[[1, W]], base=0,
                       channel_multiplier=0)
        iota_row = const_pool.tile([128, W], BF16)
        nc.vector.tensor_copy(iota_row[:], iota_ri[:])
        iota_f64i = const_pool.tile([64, 128], I32)
        nc.gpsimd.iota(iota_f64i[:], pattern=[[1, 128]], base=0,
                       channel_multiplier=0)
        iota_f64 = const_pool.tile([64, 128], F32)
        nc.vector.tensor_copy(iota_f64[:], iota_f64i[:])

        # att_i table for this core's shard, [64, NPC, HC]
        aw_sb = const_pool.tile([64, NPC, HC], BF16)

        def emit_once(rep):
            # ---- Phase A: hj table (h | att_j), batched 2 tiles / iter
            for t2 in range(T_TILES // 2 if do_build else 0):
                t = 2 * t2
                dmae = nc.sync if t2 % 2 == 0 else nc.scalar
                nf = b_in.tile([128, 2, 128], BF16, name="nf")
                dmae.dma_start(
                    nf[:].rearrange("p a b -> p (a b)"),
                    nodes_ftT[:, 128 * t:128 * (t + 2)])
                ps = b_ps.tile([128, 2, 2 * HC], F32, name="bps")
                for u in range(2):
                    nc.tensor.matmul(ps[:, u, 0:HC], nf[:, u, :], wh_sb[:],
                                     start=(u == 0), stop=False)
                    nc.tensor.matmul(ps[:, u, HC:2 * HC], nf[:, u, :],
                                     wj_sb[:], start=False, stop=(u == 1))
                st = b_st.tile([128, 2, 2 * HC], BF16, name="bst")
                if t2 % 2 == 0:
                    nc.vector.tensor_copy(st[:], ps[:])
                else:
                    nc.scalar.copy(st[:], ps[:])
                dmae.dma_start(
                    hj_table[128 * t:128 * (t + 2), :].rearrange(
                        "(a p) b -> p a b", p=128),
                    st[:])
            # att_i for own shard -> SBUF aw (both 64-col halves to part 0-63)
            for t in range(SHARD_TILES if do_build else 0):
                nf2 = b_in.tile([128, 128], BF16, tag="nf2", name="nf2")
                dmae = nc.scalar if t % 2 == 0 else nc.sync
                dmae.dma_start(nf2[:], nodes_ftT[:, 128 * t:128 * (t + 1)])
                ps2f = b_ps.tile([128, 2, 2 * HC], F32, tag="bps",
                                 name="bps2")
                ps2 = ps2f[0:64, :, 0:HC]
                nc.tensor.matmul(ps2[:, 0, :], nf2[:, 0:64], wi_sb[:],
                                 start=True, stop=True)
                nc.tensor.matmul(ps2[:, 1, :], nf2[:, 64:128], wi_sb[:],
                                 start=True, stop=True)
                if t % 2 == 0:
                    nc.scalar.copy(aw_sb[:, 2 * t:2 * t + 2, :], ps2[:])
                else:
                    nc.vector.tensor_copy(aw_sb[:, 2 * t:2 * t + 2, :],
                                          ps2[:])

            if int(os.environ.get("GAT_BARRIER", "0")):
                tc.strict_bb_all_engine_barrier()

            # ---- Phase B
            stage_n = fl_pool.tile([128, NPAIR * HC], F32, tag="sn",
                                   name="sn")
            stage_d = fl_pool.tile([128, NPAIR * HC], F32, tag="sd",
                                   name="sd")

            pair_tiles = {}
            done_slots = set()
            qn = 0
            lo_off = 0
            hi_off = 0
            boff = 0
            last_G = last_OH = last_TR = None
            prev_grp = None

            def emit_scatter(bins_p, OH_p, G_p):
                for bl in range(len(bins_p)):
                    s_bl, jj = bins_p[bl]
                    pr, half = s_bl // 2, s_bl % 2
                    if pr not in pair_tiles:
                        pair_tiles[pr] = mm_ps.tile(
                            [128, 2 * HC], F32, tag="pp", name=f"pp{pr}")
                    ps_t = pair_tiles[pr]
                    nc.tensor.matmul(
                        ps_t[HC * half:HC * half + HC, :],
                        OH_p[:, bl, :],
                        G_p[:, bl, :],
                        start=(jj == 0), stop=(jj == k_s[s_bl] - 1),
                        tile_position=(0, HC * half),
                        skip_group_check=True,
                    )
                    if jj == k_s[s_bl] - 1:
                        done_slots.add(s_bl)
                        if (s_bl ^ 1) in done_slots:
                            flush_pair(pr)

            def flush_pair(pr):
                ps_t = pair_tiles.pop(pr)
                nc.vector.tensor_copy(
                    stage_n[:, HC * pr:HC * (pr + 1)], ps_t[:, 0:HC])
                nc.vector.tensor_copy(
                    stage_d[:, HC * pr:HC * (pr + 1)], ps_t[:, HC:2 * HC])

            def emit_dma_gather(g):
                nonlocal qn, lo_off, hi_off, boff
                slots = list(range(g * GW, (g + 1) * GW))
                nlo, nhi, NB = nlo_g[g], nhi_g[g], nb_g[g]
                bins = []
                for s in slots:
                    bins += [(s, j) for j in range(lob[s])]
                for s in slots:
                    bins += [(s, lob[s] + j) for j in range(hib[s])]

                sl_t = idx_pool.tile([128, NLOMAX * 8], I16, tag="sl",
                                     name="sl")
                sl = sl_t[:, 0:nlo * 8]
                nc.sync.dma_start(sl,
                                  loidx_d[:, lo_off * 8:(lo_off + nlo) * 8])
                sh_t = idx_pool.tile([128, NHIMAX * 8], I16, tag="sh",
                                     name="sh")
                sh = sh_t[:, 0:nhi * 8]
                nc.sync.dma_start(sh,
                                  hiidx_d[:, hi_off * 8:(hi_off + nhi) * 8])
                tct_t = idx_pool.tile([128, NBMAX], BF16, tag="tc", name="tc")
                nc.scalar.dma_start(tct_t[:, 0:NB],
                                    tgtcol_d[:, boff:boff + NB])
                thl_t = idx_pool.tile([W, NBMAX], F32, tag="thl", name="thl")
                nc.scalar.dma_start(thl_t[:, 0:NB],
                                    thrlo_d[:, boff:boff + NB])
                thh_t = idx_pool.tile([W, NBMAX], F32, tag="thh", name="thh")
                nc.scalar.dma_start(thh_t[:, 0:NB],
                                    thrhi_d[:, boff:boff + NB])

                G_t = g_pool.tile([128, NBMAX, 2 * HC], BF16, tag="G",
                                  name="G")
                G = G_t[:, 0:NB, :]

                def chunked_gather(obase, table_ap, idx_tile, nbins):
                    nonlocal qn
                    parts = max(1, (nbins + CHUNK - 1) // CHUNK)
                    cuts = [nbins * i // parts for i in range(parts + 1)]
                    for a, b2 in zip(cuts[:-1], cuts[1:]):
                        if a == b2:
                            continue
                        nc.gpsimd.dma_gather(
                            out_ap=G_t[:, obase + a:obase + b2, :],
                            in_ap=table_ap,
                            idxs_ap=idx_tile[:, a * 8:b2 * 8],
                            num_idxs=(b2 - a) * 128,
                            num_idxs_reg=(b2 - a) * 128,
                            elem_size=2 * HC, queue_num=qn % 4,
                            single_packet=False,
                        )
                        qn += 1

                chunked_gather(0, hj_table[0:32768, :], sl, nlo)
                chunked_gather(nlo, hj_table[BASE1:N_PAD, :], sh, nhi)
                lo_off += nlo
                hi_off += nhi
                boff += NB
                return dict(bins=bins, NB=NB, G=G,
                            tct_t=tct_t, thl_t=thl_t, thh_t=thh_t)

            def emit_ohtr(st):
                NB = st["NB"]
                OH_t = oh_pool.tile([128, NBMAX, W], BF16, tag="OH",
                                    name="OH")
                OH = OH_t[:, 0:NB, :]
                t0 = st["tct_t"][:, 0:NB]
                in0 = AP(t0.tensor, t0.offset,
                         [list(t0.ap[0]), list(t0.ap[1]), [0, W]])
                i1 = iota_row[:]
                in1 = AP(i1.tensor, i1.offset,
                         [list(i1.ap[0]), [0, NB], list(i1.ap[1])])
                nc.vector.tensor_tensor(out=OH[:], in0=in0, in1=in1,
                                        op=ALU.is_equal)

                TR_t = tr_pool.tile([64, NBMAX, 128], BF16, tag="TR",
                                    name="TR")
                TR = TR_t[:, 0:NB, :]
                a_full = sca_pool.tile([64, NBMAX, 128], BF16, tag="sca",
                                       name="sca")
                af = a_full[:, 0:NB, :]
                io = iota_f64[:]
                io_b = AP(io.tensor, io.offset,
                          [list(io.ap[0]), [0, NB], list(io.ap[1])])
                tl_ap = st["thl_t"][:, 0:NB]
                tl_b = AP(tl_ap.tensor, tl_ap.offset,
                          [list(tl_ap.ap[0]), list(tl_ap.ap[1]), [0, 128]])
                th_ap = st["thh_t"][:, 0:NB]
                th_b = AP(th_ap.tensor, th_ap.offset,
                          [list(th_ap.ap[0]), list(th_ap.ap[1]), [0, 128]])
                nc.vector.tensor_tensor(out=af, in0=io_b, in1=tl_b,
                                        op=ALU.is_ge)
                nc.vector.tensor_tensor(out=TR, in0=io_b, in1=th_b,
                                        op=ALU.is_lt)
                nc.vector.tensor_tensor(out=TR, in0=TR, in1=af,
                                        op=ALU.mult)
                st["OH"], st["TR"] = OH, TR

            def emit_compute(st):
                if not do_dve:
                    return
                NB, G, bins = st["NB"], st["G"], st["bins"]
                OH, TR = st["OH"], st["TR"]
                S_t = s_pool.tile([128, NBMAX, HC], BF16, tag="S", name="S")
                S = S_t[:, 0:NB, :]
                for c0 in range(0, NB, 8):
                    c1 = min(c0 + 8, NB)
                    ap_t = att_ps.tile([128, 8, HC], F32, tag="att",
                                       name="att")
                    for bl in range(c0, c1):
                        s_bl = bins[bl][0]
                        nc.tensor.matmul(
                            ap_t[:, bl - c0, :], TR[:, bl, :],
                            aw_sb[:, s_bl, :], start=True, stop=True)
                    nc.vector.tensor_tensor(
                        out=S[:, c0:c1, :], in0=G[:, c0:c1, HC:2 * HC],
                        in1=ap_t[:, 0:c1 - c0, :], op=ALU.add)
                nc.vector.scalar_tensor_tensor(
                    out=S, in0=S, scalar=NEG_SLOPE, in1=S,
                    op0=ALU.mult, op1=ALU.max)
                nc.scalar.activation(G[:, :, HC:2 * HC], S, ACT.Exp)
                nc.vector.tensor_tensor(
                    out=G[:, :, 0:HC], in0=G[:, :, 0:HC],
                    in1=G[:, :, HC:2 * HC], op=ALU.mult)

            NG = NGROUPS if do_gather else 0
            st = prev = None
            for g in range(NG):
                nxt = emit_dma_gather(g)
                if do_att:
                    emit_ohtr(nxt)
                if st is not None:
                    emit_compute(st)
                    if do_mm:
                        if prev is not None:
                            emit_scatter(prev["bins"], prev["OH"],
                                         prev["G"])
                        prev = st
                st = nxt
            if st is not None:
                emit_compute(st)
                if do_mm:
                    if prev is not None:
                        emit_scatter(prev["bins"], prev["OH"], prev["G"])
                    emit_scatter(st["bins"], st["OH"], st["G"])
                last_G, last_OH, last_TR = st["G"], st.get("OH"), st.get("TR")

            # ---- Phase C
            if not do_mm:
                nc.vector.memset(stage_n[:], 0.0)
                nc.vector.memset(stage_d[:], 1.0)
            assert not pair_tiles, f"unflushed pairs: {list(pair_tiles)}"
            nc.vector.tensor_scalar_add(stage_d[:], stage_d[:], 1e-16)
            nc.scalar.activation(stage_d[:], stage_d[:], ACT.Ln)
            nc.scalar.activation(stage_d[:], stage_d[:], ACT.Exp, scale=-1.0)
            nc.vector.tensor_tensor(out=stage_n[:], in0=stage_n[:],
                                    in1=stage_d[:], op=ALU.mult)
            nc.vector.tensor_tensor(out=stage_n[:], in0=stage_n[:],
                                    in1=bias_sb[:], op=ALU.add)

            out_view = out_d[:].rearrange("(pr p) c -> p pr c", p=128)
            st_view = stage_n[:].rearrange("p (pr c) -> p pr c", c=HC)
            nc.sync.dma_start(out_view, st_view)
            return last_G, last_OH, last_TR, stage_d

        for rep in range(repeat):
            outs = emit_once(rep)
            if repeat > 1:
                tc.strict_bb_all_engine_barrier()
        last_G, last_OH, last_TR, stage_d = outs

        if debug_dump:
            NB_l = nb_g[-1]
            dump_hj = nc.dram_tensor("dump_hj", [N_PAD, 2 * HC], BF16,
                                     kind="ExternalOutput")
            dump_aw = nc.dram_tensor("dump_aw", [64, NPC * HC], BF16,
                                     kind="ExternalOutput")
            dump_sd = nc.dram_tensor("dump_sd", [128, NPAIR * HC], F32,
                                     kind="ExternalOutput")
            dump_g = nc.dram_tensor("dump_g", [128, NB_l * 2 * HC], BF16,
                                    kind="ExternalOutput")
            dump_oh = nc.dram_tensor("dump_oh", [128, NB_l * W], BF16,
                                     kind="ExternalOutput")
            dump_tr = nc.dram_tensor("dump_tr", [64, NB_l * 128], BF16,
                                     kind="ExternalOutput")
            tc.strict_bb_all_engine_barrier()
            nc.sync.dma_start(dump_hj[:], hj_table[:])
            nc.sync.dma_start(dump_aw[:],
                              aw_sb[:].rearrange("p a b -> p (a b)"))
            nc.sync.dma_start(dump_sd[:], stage_d[:])
            nc.sync.dma_start(dump_g[:],
                              last_G[:].rearrange("p a b -> p (a b)"))
            nc.sync.dma_start(dump_oh[:],
                              last_OH[:].rearrange("p a b -> p (a b)"))
            nc.sync.dma_start(dump_tr[:],
                              last_TR[:].rearrange("p a b -> p (a b)"))

    nc.compile()
    return nc


def kernel(**inputs):
    cfg = _cfg()
    in_maps, meta = _prep(inputs, cfg)
    nc = _build_program(cfg, meta)

    from concourse import bass_utils
    res = bass_utils.run_bass_kernel_spmd(
        nc, in_maps, core_ids=list(range(cfg["NC"])),
        trace=bool(int(os.environ.get("GAT_TRACE", "0"))),
    )
    kernel.last_result = res
    kernel.last_ctx = (nc, in_maps, cfg, meta)

    NSHARD = cfg["NSHARD"]
    out_new = np.concatenate(
        [res.results[c]["out"] for c in range(cfg["NC"])], axis=0)
    perm = meta["perm"]
    return out_new[perm[:cfg["N"]]]
